# revision 7
# baseline (speedup 1.0000x reference)
"""2-layer GCN (PyG GCNConv semantics) on 8 Trainium2 NeuronCores.

Strategy (1D graph partitioning, dst-sharded):
  - nodes are sharded contiguously across 8 cores (12544 padded nodes each);
  - every core builds the full layer-1 feature table h' = dinv * (x @ W1)
    (bf16, 256B rows) in its own DRAM; edges are partitioned by destination
    core so the scatter-add is local;
  - per destination window of 128 nodes, edges are gathered from the table
    with DMAGather (int16 indices force 4 source buckets of <=32768 rows),
    multiplied against a DVE-built one-hot selection matrix via TensorE
    matmuls accumulating in PSUM, and flushed to an SBUF accumulator;
  - self-loops are ordinary weight-1 edges (dinv^2*x == dinv*h' with h'
    already carrying the source dinv);
  - the epilogue applies the destination dinv, bias, ReLU;
  - layer-2 table shards are exchanged with one AllGather collective, then
    the same edge structure (resident in SBUF/DRAM) drives layer 2.
"""

import sys, os, dataclasses

for _p in ("/opt/trn_rl_repo", "/root/.axon_site/_ro/trn_rl_repo"):
    if os.path.isdir(_p) and _p not in sys.path:
        sys.path.insert(0, _p)

import numpy as np
import ml_dtypes

import concourse.bacc as bacc
import concourse.mybir as mybir
import concourse.tile as tile
from concourse.bass_utils import run_bass_kernel_spmd

F = 64           # feature width (in and out of both layers)
NCORES = 8
WIN = 128        # nodes per aggregation window (= PSUM partitions)
GCALL = 1024     # max slots per dma_gather call (HW ring limit: <=~1024 descs)
ELEM = 128       # bf16 elements per table row (64 real + 64 never-read)


@dataclasses.dataclass
class Cfg:
    n_real: int
    nwin: int                 # windows per core
    bucket_rows: int          # rows per source bucket (<= 32768)

    @property
    def shard(self):
        return self.nwin * WIN

    @property
    def npad(self):
        return self.shard * NCORES

    @property
    def nblk(self):
        return self.npad // 128

    @property
    def buck_base(self):
        return list(range(0, self.npad, self.bucket_rows))

    @property
    def nbuck(self):
        return len(self.buck_base)

    def buck_nrows(self, b):
        return min(self.bucket_rows, self.npad - self.buck_base[b])


CFG_FULL = Cfg(n_real=100000, nwin=98, bucket_rows=32768)


def _bf16(a):
    return np.asarray(a).astype(ml_dtypes.bfloat16)


def _prep(cfg, edge_index):
    """Host-side graph preprocessing. Returns (meta, per_core_inputs)."""
    src = np.asarray(edge_index[0], dtype=np.int64)
    dst = np.asarray(edge_index[1], dtype=np.int64)
    n = cfg.n_real

    deg = np.bincount(dst, minlength=n).astype(np.float64) + 1.0
    dinv = (1.0 / np.sqrt(deg)).astype(np.float32)
    dinv_pad = np.zeros(cfg.npad, np.float32)
    dinv_pad[:n] = dinv

    # append self-loops as ordinary edges
    loop = np.arange(n, dtype=np.int64)
    s_all = np.concatenate([src, loop])
    d_all = np.concatenate([dst, loop])

    core = d_all // cfg.shard
    wloc = (d_all % cfg.shard) // WIN
    buck = s_all // cfg.bucket_rows

    nb, nw = cfg.nbuck, cfg.nwin
    gid = ((core * nb + buck) * nw + wloc).astype(np.int64)
    order = np.argsort(gid, kind="stable")
    gs, ss, ds = gid[order], s_all[order], d_all[order]

    cnt = np.bincount(gs, minlength=NCORES * nb * nw).reshape(NCORES, nb, nw)
    cap = ((cnt.max(axis=0) + 127) // 128 * 128).astype(np.int64)  # [nb, nw]

    # slot offsets, bucket-major then window
    off = np.zeros((nb, nw), np.int64)
    cur = 0
    for b in range(nb):
        for w in range(nw):
            off[b, w] = cur
            cur += cap[b, w]
    S = int(cur)                       # total slots per core
    nchunks = (cap // 128).astype(np.int64)
    CTOT = int(nchunks.sum())

    # rank of each edge within its (core,bucket,window) group
    starts = np.zeros(NCORES * nb * nw + 1, np.int64)
    np.cumsum(cnt.reshape(-1), out=starts[1:])
    rank = np.arange(len(gs)) - starts[gs]
    slot = off.reshape(-1)[gs % (nb * nw)] + rank

    idx_all = np.zeros((NCORES, S), np.int16)
    dstrel_all = np.full((NCORES, S), -1.0, np.float32)
    csel = (gs // (nb * nw)).astype(np.int64)
    idx_all[csel, slot] = (ss - np.asarray(cfg.buck_base)[buck[order]]).astype(np.int16)
    dstrel_all[csel, slot] = (ds % WIN).astype(np.float32)

    # wrapped/replicated int16 index layout [128, S/16]
    assert S % 16 == 0
    idx_wrapped = np.ascontiguousarray(
        np.tile(idx_all.reshape(NCORES, S // 16, 16).transpose(0, 2, 1), (1, 8, 1))
    )
    dstrel_t = np.ascontiguousarray(
        _bf16(dstrel_all.reshape(NCORES, CTOT, 128).transpose(0, 2, 1))
    )

    dinv_all_t = np.ascontiguousarray(dinv_pad.reshape(cfg.nblk, 128).T)
    dinv_loc_t = np.ascontiguousarray(
        dinv_pad.reshape(NCORES, nw, 128).transpose(0, 2, 1)
    )

    # gather-call plan per bucket: list of (bucket, slot0, nslots)
    calls = []
    for b in range(nb):
        b0 = int(off[b, 0])
        bend = int(off[b, nw - 1] + cap[b, nw - 1])
        s0 = b0
        while s0 < bend:
            ns = min(GCALL, bend - s0)
            calls.append((b, s0, ns))
            s0 += ns

    meta = dict(
        cap=cap, nchunks=nchunks, off=off, S=S, CTOT=CTOT, calls=calls,
    )
    per_core = dict(
        idx=idx_wrapped, dstrel=dstrel_t, dinv_loc=dinv_loc_t,
    )
    shared = dict(dinv_all=dinv_all_t)
    return meta, per_core, shared


def _build(cfg, meta):
    nc = bacc.Bacc(None, target_bir_lowering=False, debug=False)
    dt = mybir.dt
    S, CTOT = meta["S"], meta["CTOT"]
    nb, nw = cfg.nbuck, cfg.nwin
    nchunks, off = meta["nchunks"], meta["off"]
    calls = meta["calls"]

    xT = nc.declare_dram_parameter("xT", [F, cfg.npad], dt.bfloat16, isOutput=False)
    idxP = nc.declare_dram_parameter("idx", [128, S // 16], dt.int16, isOutput=False)
    dstrelP = nc.declare_dram_parameter("dstrel", [128, CTOT], dt.bfloat16, isOutput=False)
    dinv_allP = nc.declare_dram_parameter("dinv_all", [128, cfg.nblk], dt.float32, isOutput=False)
    dinv_locP = nc.declare_dram_parameter("dinv_loc", [128, nw], dt.float32, isOutput=False)
    W1P = nc.declare_dram_parameter("W1", [F, F], dt.bfloat16, isOutput=False)
    W2P = nc.declare_dram_parameter("W2", [F, F], dt.bfloat16, isOutput=False)
    b1P = nc.declare_dram_parameter("b1", [128, F], dt.float32, isOutput=False)
    b2P = nc.declare_dram_parameter("b2", [128, F], dt.float32, isOutput=False)
    outP = nc.declare_dram_parameter("out", [cfg.shard, F], dt.float32, isOutput=True)

    tb1 = [
        nc.dram_tensor(f"table1_{b}", [cfg.buck_nrows(b), ELEM], dt.bfloat16)
        for b in range(nb)
    ]
    t2own = nc.dram_tensor("t2own", [cfg.shard, ELEM], dt.bfloat16)
    t2full = nc.dram_tensor("t2full", [cfg.npad, ELEM], dt.bfloat16, addr_space="Shared")

    # chunk -> (gather call index, column within call)
    chunk_call = []
    for ci, (b, s0, ns) in enumerate(calls):
        for k in range(ns // 128):
            chunk_call.append((ci, k))

    def bcast(ap, dims):
        return dataclasses.replace(ap, ap=[ap.ap[0]] + dims)

    with tile.TileContext(nc) as tc:
        with tc.tile_pool(name="meta", bufs=1) as mp, \
             tc.tile_pool(name="xs", bufs=3) as xs, \
             tc.tile_pool(name="stg", bufs=3) as stg, \
             tc.tile_pool(name="gp", bufs=3) as gp, \
             tc.tile_pool(name="sp", bufs=3) as sp, \
             tc.tile_pool(name="ip", bufs=3) as ip, \
             tc.tile_pool(name="ep", bufs=4) as ep, \
             tc.tile_pool(name="x2p", bufs=3) as x2p, \
             tc.tile_pool(name="psA", bufs=4, space="PSUM") as psA, \
             tc.tile_pool(name="psT", bufs=2, space="PSUM") as psT, \
             tc.tile_pool(name="psR", bufs=2, space="PSUM") as psR:

            # ---------------- resident constants / metadata ----------------
            dstrel_t = mp.tile([128, CTOT], dt.bfloat16)
            nc.sync.dma_start(out=dstrel_t[:], in_=dstrelP[:])
            dinv_all_t = mp.tile([128, cfg.nblk], dt.float32)
            nc.sync.dma_start(out=dinv_all_t[:], in_=dinv_allP[:])
            dinv_loc_t = mp.tile([128, nw], dt.float32)
            nc.sync.dma_start(out=dinv_loc_t[:], in_=dinv_locP[:])
            W1_t = mp.tile([F, F], dt.bfloat16)
            nc.sync.dma_start(out=W1_t[:], in_=W1P[:])
            W2_t = mp.tile([F, F], dt.bfloat16)
            nc.sync.dma_start(out=W2_t[:], in_=W2P[:])
            b1_t = mp.tile([128, F], dt.float32)
            nc.sync.dma_start(out=b1_t[:], in_=b1P[:])
            b2_t = mp.tile([128, F], dt.float32)
            nc.sync.dma_start(out=b2_t[:], in_=b2P[:])
            iota_i = mp.tile([128, 128], dt.int32)
            nc.gpsimd.iota(iota_i[:], pattern=[[1, 128]], base=0, channel_multiplier=0)
            iota_b = mp.tile([128, 128], dt.bfloat16)
            nc.vector.tensor_copy(out=iota_b[:], in_=iota_i[:])
            acc = mp.tile([128, nw * F], dt.float32)

            # identity for PE transpose: ident[p, j] = (j == p)
            pidx_i = mp.tile([128, 1], dt.int32)
            nc.gpsimd.iota(pidx_i[:], pattern=[[1, 1]], base=0, channel_multiplier=1)
            pidx_b = mp.tile([128, 1], dt.float32)
            nc.vector.tensor_copy(out=pidx_b[:], in_=pidx_i[:])
            ident = mp.tile([128, 128], dt.bfloat16)
            nc.vector.tensor_scalar(
                out=ident[:], in0=iota_b[:], scalar1=pidx_b[:, 0:1], scalar2=None,
                op0=mybir.AluOpType.is_equal,
            )

            # ---------------- layer-1 table build ----------------
            def build_table_block(blk, xt_ap, psum_pool, stage, col):
                """one 128-node block: matmul + scaled bf16 evacuation"""
                pt = psum_pool.tile([128, F], dt.float32, tag="t2")
                nc.tensor.matmul(out=pt[:], lhsT=xt_ap, rhs=W1_t[:], start=True, stop=True)
                nc.scalar.activation(
                    out=stage[:, col * 128 : col * 128 + F], in_=pt[:],
                    func=mybir.ActivationFunctionType.Copy,
                    scale=dinv_all_t[:, blk : blk + 1],
                )

            XGRP = 16   # node-blocks per xT stream tile
            WGRP = min(8, cfg.bucket_rows // 128)   # node-blocks per table-write DMA
            for g0 in range(0, cfg.nblk, XGRP):
                gn = min(XGRP, cfg.nblk - g0)
                xt = xs.tile([F, XGRP * 128], dt.bfloat16, tag="xt")
                nc.sync.dma_start(out=xt[:, : gn * 128], in_=xT[:, g0 * 128 : (g0 + gn) * 128])
                for w0 in range(0, gn, WGRP):
                    wn = min(WGRP, gn - w0)
                    stage = stg.tile([128, WGRP * 128], dt.bfloat16, tag="st")
                    for j in range(wn):
                        blk = g0 + w0 + j
                        build_table_block(blk, xt[:, (w0 + j) * 128 : (w0 + j + 1) * 128], psT, stage, j)
                    row0 = (g0 + w0) * 128
                    b = row0 // cfg.bucket_rows
                    rb = row0 - cfg.buck_base[b]
                    nc.sync.dma_start(
                        out=tb1[b][rb : rb + wn * 128, :].rearrange("(k p) e -> p k e", p=128),
                        in_=stage[:, : wn * 128].rearrange("p (k e) -> p k e", e=ELEM),
                    )

            # ---------------- aggregation (shared for both layers) ---------
            def aggregate(layer, table_aps):
                """table_aps: per-bucket DRAM APs of the gather table"""
                SGRP = 16   # chunks per one-hot build
                chunk = 0
                cur_call = -1
                g_t = None
                s_t = None
                s_base = -1
                for b in range(nb):
                    for w in range(nw):
                        nch = int(nchunks[b, w])
                        if nch == 0:
                            continue
                        pw = psA.tile([128, F], dt.float32, tag="agg")
                        for k in range(nch):
                            ci, col = chunk_call[chunk]
                            if ci != cur_call:
                                cb, s0, ns = calls[ci]
                                it = ip.tile([128, GCALL // 16], dt.int16, tag="idx")
                                nc.sync.dma_start(
                                    out=it[:, : ns // 16],
                                    in_=idxP[:, s0 // 16 : (s0 + ns) // 16],
                                )
                                g_t = gp.tile([128, GCALL // 128, ELEM], dt.bfloat16, tag="g")
                                nc.gpsimd.dma_gather(
                                    out_ap=g_t[:, : ns // 128, :],
                                    in_ap=table_aps[cb],
                                    idxs_ap=it[:, : ns // 16],
                                    num_idxs=ns,
                                    num_idxs_reg=ns,
                                    elem_size=ELEM,
                                )
                                cur_call = ci
                            if chunk >= s_base + SGRP or s_t is None:
                                s_base = chunk
                                sn = min(SGRP, CTOT - s_base)
                                s_t = sp.tile([128, SGRP, 128], dt.bfloat16, tag="s")
                                nc.vector.tensor_tensor(
                                    out=s_t[:, :sn, :],
                                    in0=bcast(iota_b[:, :], [[0, sn], [1, 128]]),
                                    in1=bcast(dstrel_t[:, s_base : s_base + sn], [[1, sn], [0, 128]]),
                                    op=mybir.AluOpType.is_equal,
                                )
                            nc.tensor.matmul(
                                out=pw[:],
                                lhsT=s_t[:, chunk - s_base, :],
                                rhs=g_t[:, col, 0:F],
                                start=(k == 0),
                                stop=(k == nch - 1),
                            )
                            chunk += 1
                        aw = acc[:, w * F : (w + 1) * F]
                        if b == 0 or nchunks[:b, w].sum() == 0:
                            nc.vector.tensor_copy(out=aw, in_=pw[:])
                        else:
                            nc.vector.tensor_tensor(out=aw, in0=aw, in1=pw[:], op=mybir.AluOpType.add)

            tb1_aps = [tb1[b][:] for b in range(nb)]
            aggregate(1, tb1_aps)

            # ---------------- layer-1 epilogue + table-2 build -------------
            for w in range(nw):
                aw = acc[:, w * F : (w + 1) * F]
                t1 = ep.tile([128, F], dt.float32, tag="e1")
                nc.vector.tensor_scalar(
                    out=t1[:], in0=aw, scalar1=dinv_loc_t[:, w : w + 1], scalar2=None,
                    op0=mybir.AluOpType.mult,
                )
                nc.vector.tensor_tensor(out=t1[:], in0=t1[:], in1=b1_t[:], op=mybir.AluOpType.add)
                x2 = ep.tile([128, F], dt.bfloat16, tag="x2")
                nc.scalar.activation(out=x2[:], in_=t1[:], func=mybir.ActivationFunctionType.Relu)
                # transpose to feature-major for the table matmul
                ptr = psR.tile([F, 128], dt.bfloat16, tag="tr")
                nc.tensor.transpose(out=ptr[:], in_=x2[:], identity=ident[:])
                x2t = x2p.tile([F, 128], dt.bfloat16, tag="x2t")
                nc.scalar.activation(out=x2t[:], in_=ptr[:], func=mybir.ActivationFunctionType.Copy)
                pt2 = psT.tile([128, F], dt.float32, tag="t2")
                nc.tensor.matmul(out=pt2[:], lhsT=x2t[:], rhs=W2_t[:], start=True, stop=True)
                st2 = stg.tile([128, 128], dt.bfloat16, tag="st2")
                nc.scalar.activation(
                    out=st2[:, :F], in_=pt2[:],
                    func=mybir.ActivationFunctionType.Copy,
                    scale=dinv_loc_t[:, w : w + 1],
                )
                nc.sync.dma_start(
                    out=t2own[w * 128 : (w + 1) * 128, :].rearrange("(k p) e -> p k e", p=128),
                    in_=st2[:].rearrange("p (k e) -> p k e", e=ELEM),
                )

            # ---------------- exchange table-2 shards ----------------------
            nc.gpsimd.collective_compute(
                "AllGather",
                mybir.AluOpType.bypass,
                ins=[t2own[:]],
                outs=[t2full[:]],
                replica_groups=[list(range(NCORES))],
            )

            # ---------------- layer 2 --------------------------------------
            tb2_aps = [
                t2full[cfg.buck_base[b] : cfg.buck_base[b] + cfg.buck_nrows(b), :]
                for b in range(nb)
            ]
            aggregate(2, tb2_aps)

            for w in range(nw):
                aw = acc[:, w * F : (w + 1) * F]
                t1 = ep.tile([128, F], dt.float32, tag="e1")
                nc.vector.tensor_scalar(
                    out=t1[:], in0=aw, scalar1=dinv_loc_t[:, w : w + 1], scalar2=None,
                    op0=mybir.AluOpType.mult,
                )
                nc.vector.tensor_tensor(out=t1[:], in0=t1[:], in1=b2_t[:], op=mybir.AluOpType.add)
                o = ep.tile([128, F], dt.float32, tag="o")
                nc.scalar.activation(out=o[:], in_=t1[:], func=mybir.ActivationFunctionType.Relu)
                nc.sync.dma_start(
                    out=outP[w * 128 : (w + 1) * 128, :].rearrange("(k p) e -> p k e", p=128),
                    in_=o[:].rearrange("p (k e) -> p k e", e=F),
                )

    nc.compile()
    return nc


_CACHE = {}


def _get_program(cfg, edge_index):
    key = hash(np.asarray(edge_index).tobytes()) ^ hash((cfg.n_real, cfg.nwin, cfg.bucket_rows))
    if key not in _CACHE:
        meta, per_core, shared = _prep(cfg, edge_index)
        nc = _build(cfg, meta)
        _CACHE[key] = (nc, meta, per_core, shared)
    return _CACHE[key]


def _run(cfg, x, edge_index, W1, b1, W2, b2):
    nc, meta, per_core, shared = _get_program(cfg, edge_index)

    xpadT = np.zeros((F, cfg.npad), np.float32)
    xpadT[:, : cfg.n_real] = np.asarray(x, np.float32).T
    xT_b = _bf16(xpadT)
    in_maps = []
    for r in range(NCORES):
        in_maps.append({
            "xT": xT_b,
            "idx": per_core["idx"][r],
            "dstrel": per_core["dstrel"][r],
            "dinv_all": shared["dinv_all"],
            "dinv_loc": per_core["dinv_loc"][r],
            "W1": _bf16(W1),
            "W2": _bf16(W2),
            "b1": np.ascontiguousarray(np.tile(np.asarray(b1, np.float32)[None, :], (128, 1))),
            "b2": np.ascontiguousarray(np.tile(np.asarray(b2, np.float32)[None, :], (128, 1))),
        })
    res = run_bass_kernel_spmd(nc, in_maps, list(range(NCORES)))
    out = np.concatenate([res.results[r]["out"] for r in range(NCORES)], axis=0)
    return out[: cfg.n_real]


def kernel(x, edge_index, W1, b1, W2, b2):
    return _run(CFG_FULL, x, edge_index, W1, b1, W2, b2)


# revision 8
# speedup vs baseline: 1.0898x; 1.0898x over previous
"""2-layer GCN (PyG GCNConv semantics) on 8 Trainium2 NeuronCores.

Strategy (1D graph partitioning, dst-sharded):
  - nodes are sharded contiguously across 8 cores (12544 padded nodes each);
  - every core builds the full layer-1 feature table h' = dinv * (x @ W1)
    (bf16, 256B rows) in its own DRAM; edges are partitioned by destination
    core so the scatter-add is local;
  - per destination window of 128 nodes, edges are gathered from the table
    with DMAGather (int16 indices force 4 source buckets of <=32768 rows),
    multiplied against a DVE-built one-hot selection matrix via TensorE
    matmuls accumulating in PSUM, and flushed to an SBUF accumulator;
  - self-loops are ordinary weight-1 edges (dinv^2*x == dinv*h' with h'
    already carrying the source dinv);
  - the epilogue applies the destination dinv, bias, ReLU;
  - layer-2 table shards are exchanged with one AllGather collective, then
    the same edge structure (resident in SBUF/DRAM) drives layer 2.
"""

import sys, os, dataclasses

for _p in ("/opt/trn_rl_repo", "/root/.axon_site/_ro/trn_rl_repo"):
    if os.path.isdir(_p) and _p not in sys.path:
        sys.path.insert(0, _p)

import numpy as np
import ml_dtypes

import concourse.bacc as bacc
import concourse.mybir as mybir
import concourse.tile as tile
from concourse.bass_utils import run_bass_kernel_spmd

F = 64           # feature width (in and out of both layers)
NCORES = 8
WIN = 128        # nodes per aggregation window (= PSUM partitions)
GCALL = 1024     # max slots per dma_gather call (HW ring limit: <=~1024 descs)
ELEM = 128       # bf16 elements per table row (64 real + 64 never-read)


@dataclasses.dataclass
class Cfg:
    n_real: int
    nwin: int                 # windows per core
    bucket_rows: int          # rows per source bucket (<= 32768)

    @property
    def shard(self):
        return self.nwin * WIN

    @property
    def npad(self):
        return self.shard * NCORES

    @property
    def nblk(self):
        return self.npad // 128

    @property
    def buck_base(self):
        return list(range(0, self.npad, self.bucket_rows))

    @property
    def nbuck(self):
        return len(self.buck_base)

    def buck_nrows(self, b):
        return min(self.bucket_rows, self.npad - self.buck_base[b])


CFG_FULL = Cfg(n_real=100000, nwin=98, bucket_rows=32768)


def _bf16(a):
    return np.asarray(a).astype(ml_dtypes.bfloat16)


def _prep(cfg, edge_index):
    """Host-side graph preprocessing. Returns (meta, per_core_inputs)."""
    src = np.asarray(edge_index[0], dtype=np.int64)
    dst = np.asarray(edge_index[1], dtype=np.int64)
    n = cfg.n_real

    deg = np.bincount(dst, minlength=n).astype(np.float64) + 1.0
    dinv = (1.0 / np.sqrt(deg)).astype(np.float32)
    dinv_pad = np.zeros(cfg.npad, np.float32)
    dinv_pad[:n] = dinv

    # append self-loops as ordinary edges
    loop = np.arange(n, dtype=np.int64)
    s_all = np.concatenate([src, loop])
    d_all = np.concatenate([dst, loop])

    core = d_all // cfg.shard
    wloc = (d_all % cfg.shard) // WIN
    buck = s_all // cfg.bucket_rows

    nb, nw = cfg.nbuck, cfg.nwin
    gid = ((core * nb + buck) * nw + wloc).astype(np.int64)
    order = np.argsort(gid, kind="stable")
    gs, ss, ds = gid[order], s_all[order], d_all[order]

    cnt = np.bincount(gs, minlength=NCORES * nb * nw).reshape(NCORES, nb, nw)
    cap = ((cnt.max(axis=0) + 127) // 128 * 128).astype(np.int64)  # [nb, nw]

    # slot offsets, bucket-major then window
    off = np.zeros((nb, nw), np.int64)
    cur = 0
    for b in range(nb):
        for w in range(nw):
            off[b, w] = cur
            cur += cap[b, w]
    S = int(cur)                       # total slots per core
    nchunks = (cap // 128).astype(np.int64)
    CTOT = int(nchunks.sum())

    # rank of each edge within its (core,bucket,window) group
    starts = np.zeros(NCORES * nb * nw + 1, np.int64)
    np.cumsum(cnt.reshape(-1), out=starts[1:])
    rank = np.arange(len(gs)) - starts[gs]
    slot = off.reshape(-1)[gs % (nb * nw)] + rank

    idx_all = np.zeros((NCORES, S), np.int16)
    dstrel_all = np.full((NCORES, S), -1.0, np.float32)
    csel = (gs // (nb * nw)).astype(np.int64)
    idx_all[csel, slot] = (ss - np.asarray(cfg.buck_base)[buck[order]]).astype(np.int16)
    dstrel_all[csel, slot] = (ds % WIN).astype(np.float32)

    # wrapped/replicated int16 index layout [128, S/16]
    assert S % 16 == 0
    idx_wrapped = np.ascontiguousarray(
        np.tile(idx_all.reshape(NCORES, S // 16, 16).transpose(0, 2, 1), (1, 8, 1))
    )
    dstrel_t = np.ascontiguousarray(
        _bf16(dstrel_all.reshape(NCORES, CTOT, 128).transpose(0, 2, 1))
    )

    dinv_all_t = np.ascontiguousarray(dinv_pad.reshape(cfg.nblk, 128).T)
    dinv_loc_t = np.ascontiguousarray(
        dinv_pad.reshape(NCORES, nw, 128).transpose(0, 2, 1)
    )

    # gather-call plan per bucket: list of (bucket, slot0, nslots)
    calls = []
    for b in range(nb):
        b0 = int(off[b, 0])
        bend = int(off[b, nw - 1] + cap[b, nw - 1])
        s0 = b0
        while s0 < bend:
            ns = min(GCALL, bend - s0)
            calls.append((b, s0, ns))
            s0 += ns

    meta = dict(
        cap=cap, nchunks=nchunks, off=off, S=S, CTOT=CTOT, calls=calls,
    )
    per_core = dict(
        idx=idx_wrapped, dstrel=dstrel_t, dinv_loc=dinv_loc_t,
    )
    shared = dict(dinv_all=dinv_all_t)
    return meta, per_core, shared


def _build(cfg, meta):
    nc = bacc.Bacc(None, target_bir_lowering=False, debug=False)
    dt = mybir.dt
    S, CTOT = meta["S"], meta["CTOT"]
    nb, nw = cfg.nbuck, cfg.nwin
    nchunks, off = meta["nchunks"], meta["off"]
    calls = meta["calls"]

    xT = nc.declare_dram_parameter("xT", [F, cfg.npad], dt.bfloat16, isOutput=False)
    idxP = nc.declare_dram_parameter("idx", [128, S // 16], dt.int16, isOutput=False)
    dstrelP = nc.declare_dram_parameter("dstrel", [128, CTOT], dt.bfloat16, isOutput=False)
    dinv_allP = nc.declare_dram_parameter("dinv_all", [128, cfg.nblk], dt.float32, isOutput=False)
    dinv_locP = nc.declare_dram_parameter("dinv_loc", [128, nw], dt.float32, isOutput=False)
    W1P = nc.declare_dram_parameter("W1", [F, F], dt.bfloat16, isOutput=False)
    W2P = nc.declare_dram_parameter("W2", [F, F], dt.bfloat16, isOutput=False)
    b1P = nc.declare_dram_parameter("b1", [128, F], dt.float32, isOutput=False)
    b2P = nc.declare_dram_parameter("b2", [128, F], dt.float32, isOutput=False)
    outP = nc.declare_dram_parameter("out", [cfg.shard, F], dt.float32, isOutput=True)

    tb1 = [
        nc.dram_tensor(f"table1_{b}", [cfg.buck_nrows(b), ELEM], dt.bfloat16)
        for b in range(nb)
    ]
    t2own = nc.dram_tensor("t2own", [cfg.shard, ELEM], dt.bfloat16)
    t2full = nc.dram_tensor("t2full", [cfg.npad, ELEM], dt.bfloat16, addr_space="Shared")

    # chunk -> (gather call index, column within call)
    chunk_call = []
    for ci, (b, s0, ns) in enumerate(calls):
        for k in range(ns // 128):
            chunk_call.append((ci, k))

    def bcast(ap, dims):
        return dataclasses.replace(ap, ap=[ap.ap[0]] + dims)

    with tile.TileContext(nc) as tc:
        with tc.tile_pool(name="meta", bufs=1) as mp, \
             tc.tile_pool(name="xs", bufs=3) as xs, \
             tc.tile_pool(name="stg", bufs=3) as stg, \
             tc.tile_pool(name="gp", bufs=3) as gp, \
             tc.tile_pool(name="sp", bufs=3) as sp, \
             tc.tile_pool(name="ip", bufs=3) as ip, \
             tc.tile_pool(name="ep", bufs=4) as ep, \
             tc.tile_pool(name="x2p", bufs=3) as x2p, \
             tc.tile_pool(name="psA", bufs=4, space="PSUM") as psA, \
             tc.tile_pool(name="psT", bufs=2, space="PSUM") as psT, \
             tc.tile_pool(name="psR", bufs=2, space="PSUM") as psR:

            # ---------------- resident constants / metadata ----------------
            dstrel_t = mp.tile([128, CTOT], dt.bfloat16)
            nc.sync.dma_start(out=dstrel_t[:], in_=dstrelP[:])
            dinv_all_t = mp.tile([128, cfg.nblk], dt.float32)
            nc.sync.dma_start(out=dinv_all_t[:], in_=dinv_allP[:])
            dinv_loc_t = mp.tile([128, nw], dt.float32)
            nc.sync.dma_start(out=dinv_loc_t[:], in_=dinv_locP[:])
            W1_t = mp.tile([F, F], dt.bfloat16)
            nc.sync.dma_start(out=W1_t[:], in_=W1P[:])
            W2_t = mp.tile([F, F], dt.bfloat16)
            nc.sync.dma_start(out=W2_t[:], in_=W2P[:])
            b1_t = mp.tile([128, F], dt.float32)
            nc.sync.dma_start(out=b1_t[:], in_=b1P[:])
            b2_t = mp.tile([128, F], dt.float32)
            nc.sync.dma_start(out=b2_t[:], in_=b2P[:])
            iota_i = mp.tile([128, 128], dt.int32)
            nc.gpsimd.iota(iota_i[:], pattern=[[1, 128]], base=0, channel_multiplier=0)
            iota_b = mp.tile([128, 128], dt.bfloat16)
            nc.vector.tensor_copy(out=iota_b[:], in_=iota_i[:])
            acc = mp.tile([128, nw * F], dt.float32)

            # identity for PE transpose: ident[p, j] = (j == p)
            pidx_i = mp.tile([128, 1], dt.int32)
            nc.gpsimd.iota(pidx_i[:], pattern=[[1, 1]], base=0, channel_multiplier=1)
            pidx_b = mp.tile([128, 1], dt.float32)
            nc.vector.tensor_copy(out=pidx_b[:], in_=pidx_i[:])
            ident = mp.tile([128, 128], dt.bfloat16)
            nc.vector.tensor_scalar(
                out=ident[:], in0=iota_b[:], scalar1=pidx_b[:, 0:1], scalar2=None,
                op0=mybir.AluOpType.is_equal,
            )

            # ---------------- layer-1 table build ----------------
            def build_table_block(blk, xt_ap, psum_pool, stage, col):
                """one 128-node block: matmul + scaled bf16 evacuation"""
                pt = psum_pool.tile([128, F], dt.float32, tag="t2")
                nc.tensor.matmul(out=pt[:], lhsT=xt_ap, rhs=W1_t[:], start=True, stop=True)
                nc.scalar.activation(
                    out=stage[:, col * 128 : col * 128 + F], in_=pt[:],
                    func=mybir.ActivationFunctionType.Copy,
                    scale=dinv_all_t[:, blk : blk + 1],
                )

            XGRP = 16   # node-blocks per xT stream tile
            WGRP = min(8, cfg.bucket_rows // 128)   # node-blocks per table-write DMA
            for g0 in range(0, cfg.nblk, XGRP):
                gn = min(XGRP, cfg.nblk - g0)
                xt = xs.tile([F, XGRP * 128], dt.bfloat16, tag="xt")
                nc.sync.dma_start(out=xt[:, : gn * 128], in_=xT[:, g0 * 128 : (g0 + gn) * 128])
                for w0 in range(0, gn, WGRP):
                    wn = min(WGRP, gn - w0)
                    stage = stg.tile([128, WGRP * 128], dt.bfloat16, tag="st")
                    for j in range(wn):
                        blk = g0 + w0 + j
                        build_table_block(blk, xt[:, (w0 + j) * 128 : (w0 + j + 1) * 128], psT, stage, j)
                    row0 = (g0 + w0) * 128
                    b = row0 // cfg.bucket_rows
                    rb = row0 - cfg.buck_base[b]
                    nc.sync.dma_start(
                        out=tb1[b][rb : rb + wn * 128, :].rearrange("(k p) e -> p k e", p=128),
                        in_=stage[:, : wn * 128].rearrange("p (k e) -> p k e", e=ELEM),
                    )

            # ---------------- aggregation (shared for both layers) ---------
            def aggregate(layer, table_aps):
                """table_aps: per-bucket DRAM APs of the gather table"""
                SGRP = 16   # chunks per one-hot build
                chunk = 0
                cur_call = -1
                g_t = None
                s_t = None
                s_base = -1
                for b in range(nb):
                    for w in range(nw):
                        nch = int(nchunks[b, w])
                        if nch == 0:
                            continue
                        pw = psA.tile([128, F], dt.float32, tag="agg")
                        for k in range(nch):
                            ci, col = chunk_call[chunk]
                            if ci != cur_call:
                                cb, s0, ns = calls[ci]
                                it = ip.tile([128, GCALL // 16], dt.int16, tag="idx")
                                nc.sync.dma_start(
                                    out=it[:, : ns // 16],
                                    in_=idxP[:, s0 // 16 : (s0 + ns) // 16],
                                )
                                g_t = gp.tile([128, GCALL // 128, ELEM], dt.bfloat16, tag="g")
                                nc.gpsimd.dma_gather(
                                    out_ap=g_t[:, : ns // 128, :],
                                    in_ap=table_aps[cb],
                                    idxs_ap=it[:, : ns // 16],
                                    num_idxs=ns,
                                    num_idxs_reg=ns,
                                    elem_size=ELEM,
                                )
                                cur_call = ci
                            if chunk >= s_base + SGRP or s_t is None:
                                s_base = chunk
                                sn = min(SGRP, CTOT - s_base)
                                s_t = sp.tile([128, SGRP, 128], dt.bfloat16, tag="s")
                                nc.vector.tensor_tensor(
                                    out=s_t[:, :sn, :],
                                    in0=bcast(iota_b[:, :], [[0, sn], [1, 128]]),
                                    in1=bcast(dstrel_t[:, s_base : s_base + sn], [[1, sn], [0, 128]]),
                                    op=mybir.AluOpType.is_equal,
                                )
                            nc.tensor.matmul(
                                out=pw[:],
                                lhsT=s_t[:, chunk - s_base, :],
                                rhs=g_t[:, col, 0:F],
                                start=(k == 0),
                                stop=(k == nch - 1),
                            )
                            chunk += 1
                        aw = acc[:, w * F : (w + 1) * F]
                        if b == 0 or nchunks[:b, w].sum() == 0:
                            nc.vector.tensor_copy(out=aw, in_=pw[:])
                        else:
                            nc.vector.tensor_tensor(out=aw, in0=aw, in1=pw[:], op=mybir.AluOpType.add)

            tb1_aps = [tb1[b][:] for b in range(nb)]
            aggregate(1, tb1_aps)

            # ---------------- layer-1 epilogue + table-2 build -------------
            for w in range(nw):
                aw = acc[:, w * F : (w + 1) * F]
                t1 = ep.tile([128, F], dt.float32, tag="e1")
                nc.vector.tensor_scalar(
                    out=t1[:], in0=aw, scalar1=dinv_loc_t[:, w : w + 1], scalar2=None,
                    op0=mybir.AluOpType.mult,
                )
                nc.vector.tensor_tensor(out=t1[:], in0=t1[:], in1=b1_t[:], op=mybir.AluOpType.add)
                x2 = ep.tile([128, F], dt.bfloat16, tag="x2")
                nc.scalar.activation(out=x2[:], in_=t1[:], func=mybir.ActivationFunctionType.Relu)
                # transpose to feature-major for the table matmul
                ptr = psR.tile([F, 128], dt.bfloat16, tag="tr")
                nc.tensor.transpose(out=ptr[:], in_=x2[:], identity=ident[:])
                x2t = x2p.tile([F, 128], dt.bfloat16, tag="x2t")
                nc.scalar.activation(out=x2t[:], in_=ptr[:], func=mybir.ActivationFunctionType.Copy)
                pt2 = psT.tile([128, F], dt.float32, tag="t2")
                nc.tensor.matmul(out=pt2[:], lhsT=x2t[:], rhs=W2_t[:], start=True, stop=True)
                st2 = stg.tile([128, 128], dt.bfloat16, tag="st2")
                nc.scalar.activation(
                    out=st2[:, :F], in_=pt2[:],
                    func=mybir.ActivationFunctionType.Copy,
                    scale=dinv_loc_t[:, w : w + 1],
                )
                nc.sync.dma_start(
                    out=t2own[w * 128 : (w + 1) * 128, :].rearrange("(k p) e -> p k e", p=128),
                    in_=st2[:].rearrange("p (k e) -> p k e", e=ELEM),
                )

            # ---------------- exchange table-2 shards ----------------------
            if os.environ.get("K_NO_COLLECTIVE"):
                # timing-only variant (TimelineSim can't model collectives):
                # fake the exchange with a local copy of the own shard
                nc.sync.dma_start(out=t2full[: cfg.shard, :], in_=t2own[:])
            else:
                nc.gpsimd.collective_compute(
                    "AllGather",
                    mybir.AluOpType.bypass,
                    ins=[t2own[:]],
                    outs=[t2full[:]],
                    replica_groups=[list(range(NCORES))],
                )

            # ---------------- layer 2 --------------------------------------
            tb2_aps = [
                t2full[cfg.buck_base[b] : cfg.buck_base[b] + cfg.buck_nrows(b), :]
                for b in range(nb)
            ]
            aggregate(2, tb2_aps)

            for w in range(nw):
                aw = acc[:, w * F : (w + 1) * F]
                t1 = ep.tile([128, F], dt.float32, tag="e1")
                nc.vector.tensor_scalar(
                    out=t1[:], in0=aw, scalar1=dinv_loc_t[:, w : w + 1], scalar2=None,
                    op0=mybir.AluOpType.mult,
                )
                nc.vector.tensor_tensor(out=t1[:], in0=t1[:], in1=b2_t[:], op=mybir.AluOpType.add)
                o = ep.tile([128, F], dt.float32, tag="o")
                nc.scalar.activation(out=o[:], in_=t1[:], func=mybir.ActivationFunctionType.Relu)
                nc.sync.dma_start(
                    out=outP[w * 128 : (w + 1) * 128, :].rearrange("(k p) e -> p k e", p=128),
                    in_=o[:].rearrange("p (k e) -> p k e", e=F),
                )

    nc.compile()
    return nc


_CACHE = {}


def _get_program(cfg, edge_index):
    key = hash(np.asarray(edge_index).tobytes()) ^ hash((cfg.n_real, cfg.nwin, cfg.bucket_rows))
    if key not in _CACHE:
        meta, per_core, shared = _prep(cfg, edge_index)
        nc = _build(cfg, meta)
        _CACHE[key] = (nc, meta, per_core, shared)
    return _CACHE[key]


def _run(cfg, x, edge_index, W1, b1, W2, b2):
    nc, meta, per_core, shared = _get_program(cfg, edge_index)

    xpadT = np.zeros((F, cfg.npad), np.float32)
    xpadT[:, : cfg.n_real] = np.asarray(x, np.float32).T
    xT_b = _bf16(xpadT)
    in_maps = []
    for r in range(NCORES):
        in_maps.append({
            "xT": xT_b,
            "idx": per_core["idx"][r],
            "dstrel": per_core["dstrel"][r],
            "dinv_all": shared["dinv_all"],
            "dinv_loc": per_core["dinv_loc"][r],
            "W1": _bf16(W1),
            "W2": _bf16(W2),
            "b1": np.ascontiguousarray(np.tile(np.asarray(b1, np.float32)[None, :], (128, 1))),
            "b2": np.ascontiguousarray(np.tile(np.asarray(b2, np.float32)[None, :], (128, 1))),
        })
    res = run_bass_kernel_spmd(nc, in_maps, list(range(NCORES)))
    out = np.concatenate([res.results[r]["out"] for r in range(NCORES)], axis=0)
    return out[: cfg.n_real]


def kernel(x, edge_index, W1, b1, W2, b2):
    return _run(CFG_FULL, x, edge_index, W1, b1, W2, b2)
